# revision 23
# baseline (speedup 1.0000x reference)
"""Causal self-attention (GQA + RoPE) on 8 Trainium2 NeuronCores.

Sharding: data-parallel over batch (2) x tensor-parallel over KV-head groups
(4).  Core c handles batch b=c//4 and KV group g=c%4 (Q heads 4g..4g+3).
Each core computes qkv^T in a transposed [d, t] layout, runs attention with
scores in S^T[k, q] orientation (no transposes needed), AllGathers y^T
across its 4-core group per 512-token chunk (pipelined with compute), then
computes the output projection for its own 512-wide output-column slice
(o-sharded, so the SPMD program is uniform).  All matmuls run in fp32r
(full-rate fp32).
"""

import sys

if "/opt/trn_rl_repo" not in sys.path:
    sys.path.insert(0, "/opt/trn_rl_repo")

import numpy as np

# Problem constants (hardcoded per contract)
B, T, C = 2, 2048, 2048
H, KV, HD = 16, 4, 128
G = 4               # TP groups (KV heads) per batch
N_CORES = 8
N_HL = H // KV      # local Q heads per core = 4
ROPE_THETA = 10000.0
TC = 512            # free-dim chunk for moving operands
NT = T // TC        # 4 t-chunks
NCT = C // 128      # 16 contraction tiles
NKT = T // 128      # 16 key tiles
SCALE = float(1.0 / np.sqrt(HD))

_COMPILED = {}


def _build(masked: bool):
    import concourse.bacc as bacc
    import concourse.tile as tile
    import concourse.mybir as mybir

    f32 = mybir.dt.float32
    f32r = mybir.dt.float32r

    nc = bacc.Bacc("TRN2", target_bir_lowering=False, debug=False,
                   num_devices=N_CORES, num_swdge_queues=4)

    bf16 = mybir.dt.bfloat16
    xT = nc.dram_tensor("xT", [C, T], bf16, kind="ExternalInput").ap()
    wqkvT = nc.dram_tensor("wqkvT", [C, (N_HL + 2) * HD], bf16,
                           kind="ExternalInput").ap()
    wpT = nc.dram_tensor("wpT", [C, TC], f32r, kind="ExternalInput").ap()
    cosT = nc.dram_tensor("cosT", [HD, T], f32, kind="ExternalInput").ap()
    sinT = nc.dram_tensor("sinT", [HD, T], f32, kind="ExternalInput").ap()
    binmask = nc.dram_tensor("binmask", [128, G * TC], bf16,
                             kind="ExternalInput").ap()
    kmask = None
    if masked:
        kmask = nc.dram_tensor("kmask", [128, NKT], f32,
                               kind="ExternalInput").ap()
    out = nc.dram_tensor("out", [T, TC], f32, kind="ExternalOutput").ap()

    with tile.TileContext(nc, pool_alloc_mode="queue") as tc, \
         nc.allow_low_precision(reason="fp32r intermediates are intentional"):
        _build_body(nc, tc, mybir, f32, f32r,
                    xT, wqkvT, wpT, cosT, sinT, binmask, kmask, out)
    nc.compile()
    return nc


def _build_body(nc, tc, mybir, f32, f32r,
                xT, wqkvT, wpT, cosT, sinT, binmask, kmask, out):
    bf16 = mybir.dt.bfloat16
    from contextlib import ExitStack
    from concourse.masks import make_identity

    AF = mybir.ActivationFunctionType
    NR = N_HL + 2  # 6 row-tiles of qkv^T (4 q heads, k, v)
    NQ = TC // 128  # 4 128-subtiles per chunk

    with ExitStack() as ctx:
        # ---- pools (all share the kernel lifetime; queue allocator) ----
        const = ctx.enter_context(tc.tile_pool(name="const", bufs=1))
        rqkv = ctx.enter_context(tc.tile_pool(name="rqkv", bufs=1))
        dram = ctx.enter_context(tc.tile_pool(name="dram", bufs=1, space="DRAM"))
        raw_pool = ctx.enter_context(tc.tile_pool(name="raw_pool", bufs=3))
        rope_tmp = ctx.enter_context(tc.tile_pool(name="rope_tmp", bufs=3))
        pt_pool = ctx.enter_context(tc.tile_pool(name="pt_pool", bufs=6))
        norm_pool = ctx.enter_context(tc.tile_pool(name="norm_pool", bufs=3))
        yt_pool = ctx.enter_context(tc.tile_pool(name="yt_pool", bufs=4))
        o_sb_pool = ctx.enter_context(tc.tile_pool(name="o_sb", bufs=3))
        # PSUM: 4 + 2 + 2 = 8 banks
        mm_psum = ctx.enter_context(
            tc.tile_pool(name="mm_psum", bufs=3, space="PSUM"))
        y_psum = ctx.enter_context(
            tc.tile_pool(name="y_psum", bufs=3, space="PSUM"))
        sum_psum = ctx.enter_context(
            tc.tile_pool(name="sum_psum", bufs=2, space="PSUM"))

        ident = const.tile([128, 128], f32)
        make_identity(nc, ident[:])
        ones_sq_f = const.tile([128, 128], f32)
        nc.vector.memset(ones_sq_f[:], 1.0)
        ones_sq = const.tile([128, 128], bf16)
        nc.vector.tensor_copy(ones_sq[:], ones_sq_f[:])
        mask_sb = const.tile([128, G * TC], bf16)
        nc.gpsimd.dma_start(mask_sb[:], binmask[:])
        kmask_sb = None
        if kmask is not None:
            kmask_sb = const.tile([128, NKT], f32)
            nc.sync.dma_start(kmask_sb[:], kmask[:])

        # resident activations (rotated q/k in [d, t] layout, v in [t, d])
        rq = [rqkv.tile([HD, T], f32r, tag=f"rq{h}", name=f"rq{h}")
              for h in range(N_HL)]
        rk = rqkv.tile([HD, T], f32r, tag="rk")
        v_sb = [rqkv.tile([128, HD], bf16, tag=f"v{i}", name=f"v{i}")
                for i in range(NKT)]

        # per-chunk AllGather bounce buffers
        ag_in = [dram.tile([G * HD, TC], bf16, tag=f"agi{j}", name=f"agi{j}")
                 for j in range(NT)]
        ag_out = [dram.tile([C, TC], bf16, tag=f"ago{j}", name=f"ago{j}")
                  for j in range(NT)]

        # ============ Phase A: qkv^T projection + RoPE + V transpose ============
        def phase_a(j, wq, xt_pool, cos_sb, sin_sb):
            xt_tiles = []
            for ct in range(NCT):
                xt_t = xt_pool.tile([128, TC], bf16, tag=f"xt{ct}",
                                    name=f"xt{ct}_{j}")
                nc.sync.dma_start(xt_t[:], xT[128 * ct:128 * (ct + 1),
                                              TC * j:TC * (j + 1)])
                xt_tiles.append(xt_t)
            for r in range(NR):
                ps = mm_psum.tile([128, TC], f32, tag="mm", name=f"qkv{r}_{j}")
                for ct in range(NCT):
                    nc.tensor.matmul(ps[:],
                                     wq[ct][:, 128 * r:128 * (r + 1)],
                                     xt_tiles[ct][:],
                                     start=(ct == 0), stop=(ct == NCT - 1))
                if r < N_HL + 1:
                    # RoPE straight off PSUM:
                    #   dst = ps*cos + rot_half(ps)*sin_signed
                    dst = (rq[r] if r < N_HL else rk)[:, TC * j:TC * (j + 1)]
                    cs = cos_sb[:, TC * j:TC * (j + 1)]
                    sn = sin_sb[:, TC * j:TC * (j + 1)]
                    t1 = rope_tmp.tile([128, TC], f32, tag="t1",
                                       name=f"t1_{r}_{j}")
                    nc.vector.tensor_mul(t1[:], ps[:], cs[:])
                    t2 = rope_tmp.tile([128, TC], f32, tag="t2",
                                       name=f"t2_{r}_{j}")
                    nc.vector.tensor_mul(t2[0:64, :], ps[64:128, :], sn[0:64, :])
                    nc.vector.tensor_mul(t2[64:128, :], ps[0:64, :], sn[64:128, :])
                    nc.vector.tensor_add(dst, t1[:], t2[:])
                else:
                    # V: evict then transpose v^T [d, t] -> v [t, d]
                    rt = raw_pool.tile([128, TC], f32, tag="raw",
                                       name=f"vraw{j}")
                    nc.vector.tensor_copy(rt[:], ps[:])
                    for q in range(NQ):
                        pt = mm_psum.tile([128, 128], f32, tag="mm",
                                          name=f"vt{j}_{q}")
                        nc.tensor.transpose(
                            pt[:], rt[:, 128 * q:128 * (q + 1)], ident[:])
                        nc.vector.tensor_copy(v_sb[j * NQ + q][:], pt[:])

        # ============ Phase B: attention for q-chunk j ============
        def phase_b(j):
            nkt = (j + 1) * NQ  # causal limit in 128-k tiles
            ps_y = {}
            ps_sum = {}
            pt = {}

            def emit_scores(h, kt):
                ps_s = mm_psum.tile([128, TC], f32, tag="mm",
                                    name=f"s{h}_{j}_{kt}")
                nc.tensor.matmul(ps_s[:],
                                 rk[:, 128 * kt:128 * (kt + 1)],
                                 rq[h][:, TC * j:TC * (j + 1)],
                                 start=True, stop=True)
                p = pt_pool.tile([128, TC], bf16, tag="pt",
                                 name=f"pt{h}_{j}_{kt}")
                nc.scalar.activation(p[:], ps_s[:], AF.Exp, scale=SCALE)
                r = kt - NQ * j
                if r >= 0:
                    nc.vector.tensor_mul(
                        p[:], p[:], mask_sb[:, TC * r:TC * (r + 1)])
                if kmask_sb is not None:
                    nc.vector.tensor_scalar_mul(
                        p[:], p[:], kmask_sb[:, kt:kt + 1])
                pt[(h, kt)] = p

            for hp in range(N_HL // 2):
                pair = (2 * hp, 2 * hp + 1)
                units = [(h, kt) for kt in range(nkt) for h in pair]
                emit_scores(*units[0])
                if len(units) > 1:
                    emit_scores(*units[1])
                for idx, (h, kt) in enumerate(units):
                    if idx + 2 < len(units):
                        emit_scores(*units[idx + 2])
                    if kt == 0:
                        ps_y[h] = y_psum.tile([HD, TC], f32, tag="y",
                                              name=f"y{h}_{j}")
                        ps_sum[h] = sum_psum.tile([128, TC], f32, tag="sum",
                                                  name=f"sum{h}_{j}")
                    p = pt.pop((h, kt))
                    nc.tensor.matmul(ps_sum[h][:], ones_sq[:], p[:],
                                     start=(kt == 0), stop=(kt == nkt - 1))
                    nc.tensor.matmul(ps_y[h][:], v_sb[kt][:], p[:],
                                     start=(kt == 0), stop=(kt == nkt - 1))
                    if kt != nkt - 1:
                        continue
                    # evict accumulators fast so the PSUM banks free up,
                    # then normalize from SBUF
                    ssb = norm_pool.tile([HD, TC], f32, tag="ssb",
                                         name=f"ss{h}_{j}")
                    nc.vector.tensor_copy(ssb[:], ps_sum[h][:])
                    ysb = norm_pool.tile([HD, TC], f32, tag="ysb",
                                         name=f"ys{h}_{j}")
                    nc.vector.tensor_copy(ysb[:], ps_y[h][:])
                    rsum = norm_pool.tile([HD, TC], f32, tag="rsum",
                                          name=f"rs{h}_{j}")
                    nc.vector.reciprocal(rsum[:], ssb[:])
                    yt_t = yt_pool.tile([HD, TC], bf16, tag="yt",
                                        name=f"yt{h}_{j}")
                    nc.vector.tensor_mul(yt_t[:], ysb[:], rsum[:])
                    nc.gpsimd.dma_start(ag_in[j][HD * h:HD * (h + 1), :],
                                        yt_t[:])

        # ============ Phase C+D: AllGather chunk + output projection ============
        def phase_c(j):
            nc.gpsimd.collective_compute(
                "AllGather",
                mybir.AluOpType.bypass,
                replica_groups=[[0, 1, 2, 3], [4, 5, 6, 7]],
                ins=[ag_in[j].opt()],
                outs=[ag_out[j].opt()],
            )

        def phase_cd(j, wp, ytf_pool):
            ytf = []
            for ct in range(NCT):
                yf = ytf_pool.tile([128, TC], bf16, tag=f"ytf{ct}",
                                   name=f"ytf{ct}_{j}")
                nc.sync.dma_start(yf[:],
                                  ag_out[j][128 * ct:128 * (ct + 1), :])
                ytf.append(yf)
            for tt in range(NQ):
                ps = mm_psum.tile([128, TC], f32, tag="mm", name=f"o{j}_{tt}")
                for ct in range(NCT):
                    nc.tensor.matmul(
                        ps[:], ytf[ct][:, 128 * tt:128 * (tt + 1)], wp[ct][:],
                        start=(ct == 0), stop=(ct == NCT - 1))
                ot = o_sb_pool.tile([128, TC], f32, tag="ot",
                                    name=f"ot{j}_{tt}")
                nc.vector.tensor_copy(ot[:], ps[:])
                nc.gpsimd.dma_start(
                    out[TC * j + 128 * tt:TC * j + 128 * (tt + 1), :], ot[:])

        with tc.tile_pool(name="wq_pool", bufs=1) as wq_pool, \
             tc.tile_pool(name="xt_pool", bufs=2) as xt_pool, \
             tc.tile_pool(name="cs_pool", bufs=1) as cs_pool:
            cos_sb = cs_pool.tile([HD, T], f32)
            sin_sb = cs_pool.tile([HD, T], f32)
            nc.gpsimd.dma_start(cos_sb[:], cosT[:])
            nc.gpsimd.dma_start(sin_sb[:], sinT[:])
            wq = [wq_pool.tile([128, NR * HD], bf16, tag=f"wq{ct}",
                               name=f"wq{ct}")
                  for ct in range(NCT)]
            for ct in range(NCT):
                nc.scalar.dma_start(wq[ct][:],
                                    wqkvT[128 * ct:128 * (ct + 1), :])
            for j in range(NT):
                phase_a(j, wq, xt_pool, cos_sb, sin_sb)

        with tc.tile_pool(name="wp_pool", bufs=1) as wp_pool, \
             tc.tile_pool(name="ytf_pool", bufs=2) as ytf_pool:
            wp = [wp_pool.tile([128, TC], bf16, tag=f"wp{ct}", name=f"wp{ct}")
                  for ct in range(NCT)]
            for ct in range(NCT):
                nc.gpsimd.dma_start(wp[ct][:], wpT[128 * ct:128 * (ct + 1), :])
            phase_b(0)
            phase_c(0)
            phase_b(1)
            phase_c(1)
            phase_b(2)
            phase_c(2)
            phase_cd(0, wp, ytf_pool)
            phase_b(3)
            phase_c(3)
            phase_cd(1, wp, ytf_pool)
            phase_cd(2, wp, ytf_pool)
            phase_cd(3, wp, ytf_pool)


def _rope_tables():
    inv_freq = 1.0 / (ROPE_THETA ** (np.arange(0, HD, 2, dtype=np.float32) / HD))
    pos = np.arange(T, dtype=np.float32)
    freqs = pos[:, None] * inv_freq[None, :]
    emb = np.concatenate([freqs, freqs], axis=-1)          # [T, HD]
    cos = np.ascontiguousarray(np.cos(emb).astype(np.float32).T)   # [HD, T]
    sin = np.ascontiguousarray(np.sin(emb).astype(np.float32).T)
    sin[:64, :] *= -1.0                                    # sign for rotate_half
    return cos, sin


def _binmask():
    kk = np.arange(128)[:, None]
    qq = np.arange(TC)[None, :]
    blocks = [(kk <= qq - 128 * r).astype(np.float32) for r in range(G)]
    return np.ascontiguousarray(np.concatenate(blocks, axis=1))  # [128, 4*512]


def kernel(x, attention_mask, w_qkv, w_proj):
    from concourse.bass_utils import run_bass_kernel_spmd

    x = np.asarray(x, dtype=np.float32)
    attention_mask = np.asarray(attention_mask, dtype=np.float32)
    w_qkv = np.asarray(w_qkv, dtype=np.float32)
    w_proj = np.asarray(w_proj, dtype=np.float32)

    masked = not bool((attention_mask == 1.0).all())
    if masked:
        attention_mask = (attention_mask != 0.0).astype(np.float32)

    if masked not in _COMPILED:
        _COMPILED[masked] = _build(masked)
    nc = _COMPILED[masked]

    import ml_dtypes
    bf = ml_dtypes.bfloat16
    cos, sin = _rope_tables()
    bm = _binmask().astype(bf)
    wpT_full = np.ascontiguousarray(w_proj.T)              # [c, o]

    in_maps = []
    for c in range(N_CORES):
        b, g = divmod(c, G)
        xT_b = np.ascontiguousarray(x[b].T).astype(bf)
        rows_q = w_qkv[512 * g:512 * (g + 1)]
        rows_k = w_qkv[H * HD + HD * g:H * HD + HD * (g + 1)]
        rows_v = w_qkv[(H + KV) * HD + HD * g:(H + KV) * HD + HD * (g + 1)]
        wqkvT_g = np.ascontiguousarray(
            np.concatenate([rows_q, rows_k, rows_v], axis=0).T).astype(bf)
        m = {"xT": xT_b,
             "wqkvT": wqkvT_g,
             "wpT": np.ascontiguousarray(wpT_full[:, TC * g:TC * (g + 1)]),
             "cosT": cos, "sinT": sin, "binmask": bm}
        if masked:
            m["kmask"] = np.ascontiguousarray(
                attention_mask[b].reshape(NKT, 128).T)
        in_maps.append(m)

    trace = bool(globals().get("_TRACE", False))
    res = run_bass_kernel_spmd(nc, in_maps, core_ids=list(range(N_CORES)),
                               trace=trace)
    globals()["_LAST_RESULT"] = res

    y = np.empty((B, T, C), dtype=np.float32)
    for c in range(N_CORES):
        b, g = divmod(c, G)
        y[b, :, TC * g:TC * (g + 1)] = res.results[c]["out"]
    return y


# revision 24
# speedup vs baseline: 1.0155x; 1.0155x over previous
"""Causal self-attention (GQA + RoPE) on 8 Trainium2 NeuronCores.

Sharding: data-parallel over batch (2) x tensor-parallel over KV-head groups
(4).  Core c handles batch b=c//4 and KV group g=c%4 (Q heads 4g..4g+3).
Each core computes qkv^T in a transposed [d, t] layout, runs attention with
scores in S^T[k, q] orientation (no transposes needed), AllGathers y^T
across its 4-core group per 512-token chunk (pipelined with compute), then
computes the output projection for its own 512-wide output-column slice
(o-sharded, so the SPMD program is uniform).  Matmuls run in fp32r for
scores and bf16 elsewhere (LDWEIGHTS hides behind 1-cycle/row bf16).
"""

import sys

if "/opt/trn_rl_repo" not in sys.path:
    sys.path.insert(0, "/opt/trn_rl_repo")

import numpy as np

# Problem constants (hardcoded per contract)
B, T, C = 2, 2048, 2048
H, KV, HD = 16, 4, 128
G = 4               # TP groups (KV heads) per batch
N_CORES = 8
N_HL = H // KV      # local Q heads per core = 4
ROPE_THETA = 10000.0
TC = 512            # free-dim chunk for moving operands
NT = T // TC        # 4 t-chunks
NCT = C // 128      # 16 contraction tiles
NKT = T // 128      # 16 key tiles
SCALE = float(1.0 / np.sqrt(HD))

_COMPILED = {}


def _build(masked: bool):
    import concourse.bacc as bacc
    import concourse.tile as tile
    import concourse.mybir as mybir

    f32 = mybir.dt.float32
    f32r = mybir.dt.float32r

    nc = bacc.Bacc("TRN2", target_bir_lowering=False, debug=False,
                   num_devices=N_CORES, num_swdge_queues=4)

    bf16 = mybir.dt.bfloat16
    xT = nc.dram_tensor("xT", [C, T], bf16, kind="ExternalInput").ap()
    wqkvT = nc.dram_tensor("wqkvT", [C, (N_HL + 2) * HD], bf16,
                           kind="ExternalInput").ap()
    wpT = nc.dram_tensor("wpT", [C, TC], f32r, kind="ExternalInput").ap()
    cosT = nc.dram_tensor("cosT", [HD, T], f32, kind="ExternalInput").ap()
    sinT = nc.dram_tensor("sinT", [HD, T], f32, kind="ExternalInput").ap()
    binmask = nc.dram_tensor("binmask", [128, G * TC], bf16,
                             kind="ExternalInput").ap()
    kmask = None
    if masked:
        kmask = nc.dram_tensor("kmask", [128, NKT], f32,
                               kind="ExternalInput").ap()
    out = nc.dram_tensor("out", [T, TC], f32, kind="ExternalOutput").ap()

    with tile.TileContext(nc, pool_alloc_mode="queue") as tc, \
         nc.allow_low_precision(reason="fp32r intermediates are intentional"):
        _build_body(nc, tc, mybir, f32, f32r,
                    xT, wqkvT, wpT, cosT, sinT, binmask, kmask, out)
    nc.compile()
    return nc


def _build_body(nc, tc, mybir, f32, f32r,
                xT, wqkvT, wpT, cosT, sinT, binmask, kmask, out):
    bf16 = mybir.dt.bfloat16
    from contextlib import ExitStack
    from concourse.masks import make_identity

    AF = mybir.ActivationFunctionType
    NR = N_HL + 2  # 6 row-tiles of qkv^T (4 q heads, k, v)
    NQ = TC // 128  # 4 128-subtiles per chunk

    with ExitStack() as ctx:
        # ---- pools (all share the kernel lifetime; queue allocator) ----
        const = ctx.enter_context(tc.tile_pool(name="const", bufs=1))
        rqkv = ctx.enter_context(tc.tile_pool(name="rqkv", bufs=1))
        dram = ctx.enter_context(tc.tile_pool(name="dram", bufs=1, space="DRAM"))
        raw_pool = ctx.enter_context(tc.tile_pool(name="raw_pool", bufs=3))
        rope_tmp = ctx.enter_context(tc.tile_pool(name="rope_tmp", bufs=3))
        pt_pool = ctx.enter_context(tc.tile_pool(name="pt_pool", bufs=6))
        norm_pool = ctx.enter_context(tc.tile_pool(name="norm_pool", bufs=3))
        yt_pool = ctx.enter_context(tc.tile_pool(name="yt_pool", bufs=4))
        o_sb_pool = ctx.enter_context(tc.tile_pool(name="o_sb", bufs=3))
        # PSUM: 4 + 2 + 2 = 8 banks
        mm_psum = ctx.enter_context(
            tc.tile_pool(name="mm_psum", bufs=4, space="PSUM"))
        y_psum = ctx.enter_context(
            tc.tile_pool(name="y_psum", bufs=2, space="PSUM"))
        sum_psum = ctx.enter_context(
            tc.tile_pool(name="sum_psum", bufs=2, space="PSUM"))

        ident = const.tile([128, 128], f32)
        make_identity(nc, ident[:])
        ones_sq_f = const.tile([128, 128], f32)
        nc.vector.memset(ones_sq_f[:], 1.0)
        ones_sq = const.tile([128, 128], bf16)
        nc.vector.tensor_copy(ones_sq[:], ones_sq_f[:])
        mask_sb = const.tile([128, G * TC], bf16)
        nc.gpsimd.dma_start(mask_sb[:], binmask[:])
        kmask_sb = None
        if kmask is not None:
            kmask_sb = const.tile([128, NKT], f32)
            nc.sync.dma_start(kmask_sb[:], kmask[:])

        # resident activations (rotated q/k in [d, t] layout, v in [t, d])
        rq = [rqkv.tile([HD, T], f32r, tag=f"rq{h}", name=f"rq{h}")
              for h in range(N_HL)]
        rk = rqkv.tile([HD, T], f32r, tag="rk")
        v_sb = [rqkv.tile([128, HD], bf16, tag=f"v{i}", name=f"v{i}")
                for i in range(NKT)]

        # per-chunk AllGather bounce buffers
        ag_in = [dram.tile([G * HD, TC], bf16, tag=f"agi{j}", name=f"agi{j}")
                 for j in range(NT)]
        ag_out = [dram.tile([C, TC], bf16, tag=f"ago{j}", name=f"ago{j}")
                  for j in range(NT)]

        # ============ Phase A: qkv^T projection + RoPE + V transpose ============
        def phase_a(j, wq, xt_pool, cos_sb, sin_sb):
            xt_tiles = []
            for ct in range(NCT):
                xt_t = xt_pool.tile([128, TC], bf16, tag=f"xt{ct}",
                                    name=f"xt{ct}_{j}")
                nc.sync.dma_start(xt_t[:], xT[128 * ct:128 * (ct + 1),
                                              TC * j:TC * (j + 1)])
                xt_tiles.append(xt_t)
            for r in range(NR):
                ps = mm_psum.tile([128, TC], f32, tag="mm", name=f"qkv{r}_{j}")
                for ct in range(NCT):
                    nc.tensor.matmul(ps[:],
                                     wq[ct][:, 128 * r:128 * (r + 1)],
                                     xt_tiles[ct][:],
                                     start=(ct == 0), stop=(ct == NCT - 1))
                if r < N_HL + 1:
                    # RoPE straight off PSUM:
                    #   dst = ps*cos + rot_half(ps)*sin_signed
                    dst = (rq[r] if r < N_HL else rk)[:, TC * j:TC * (j + 1)]
                    cs = cos_sb[:, TC * j:TC * (j + 1)]
                    sn = sin_sb[:, TC * j:TC * (j + 1)]
                    t1 = rope_tmp.tile([128, TC], f32, tag="t1",
                                       name=f"t1_{r}_{j}")
                    nc.vector.tensor_mul(t1[:], ps[:], cs[:])
                    t2 = rope_tmp.tile([128, TC], f32, tag="t2",
                                       name=f"t2_{r}_{j}")
                    nc.vector.tensor_mul(t2[0:64, :], ps[64:128, :], sn[0:64, :])
                    nc.vector.tensor_mul(t2[64:128, :], ps[0:64, :], sn[64:128, :])
                    nc.vector.tensor_add(dst, t1[:], t2[:])
                else:
                    # V: evict then transpose v^T [d, t] -> v [t, d]
                    rt = raw_pool.tile([128, TC], f32, tag="raw",
                                       name=f"vraw{j}")
                    nc.vector.tensor_copy(rt[:], ps[:])
                    for q in range(NQ):
                        pt = mm_psum.tile([128, 128], f32, tag="mm",
                                          name=f"vt{j}_{q}")
                        nc.tensor.transpose(
                            pt[:], rt[:, 128 * q:128 * (q + 1)], ident[:])
                        nc.vector.tensor_copy(v_sb[j * NQ + q][:], pt[:])

        # ============ Phase B: attention for q-chunk j ============
        def phase_b(j):
            nkt = (j + 1) * NQ  # causal limit in 128-k tiles
            ps_y = {}
            ps_sum = {}
            pt = {}

            def emit_scores(h, kt):
                ps_s = mm_psum.tile([128, TC], f32, tag="mm",
                                    name=f"s{h}_{j}_{kt}")
                nc.tensor.matmul(ps_s[:],
                                 rk[:, 128 * kt:128 * (kt + 1)],
                                 rq[h][:, TC * j:TC * (j + 1)],
                                 start=True, stop=True)
                p = pt_pool.tile([128, TC], bf16, tag="pt",
                                 name=f"pt{h}_{j}_{kt}")
                nc.scalar.activation(p[:], ps_s[:], AF.Exp, scale=SCALE)
                r = kt - NQ * j
                if r >= 0:
                    nc.vector.tensor_mul(
                        p[:], p[:], mask_sb[:, TC * r:TC * (r + 1)])
                if kmask_sb is not None:
                    nc.vector.tensor_scalar_mul(
                        p[:], p[:], kmask_sb[:, kt:kt + 1])
                pt[(h, kt)] = p

            for hp in range(N_HL // 2):
                pair = (2 * hp, 2 * hp + 1)
                units = [(h, kt) for kt in range(nkt) for h in pair]
                emit_scores(*units[0])
                if len(units) > 1:
                    emit_scores(*units[1])
                for idx, (h, kt) in enumerate(units):
                    if idx + 2 < len(units):
                        emit_scores(*units[idx + 2])
                    if kt == 0:
                        ps_y[h] = y_psum.tile([HD, TC], f32, tag="y",
                                              name=f"y{h}_{j}")
                        ps_sum[h] = sum_psum.tile([128, TC], f32, tag="sum",
                                                  name=f"sum{h}_{j}")
                    p = pt.pop((h, kt))
                    nc.tensor.matmul(ps_sum[h][:], ones_sq[:], p[:],
                                     start=(kt == 0), stop=(kt == nkt - 1))
                    nc.tensor.matmul(ps_y[h][:], v_sb[kt][:], p[:],
                                     start=(kt == 0), stop=(kt == nkt - 1))
                    if kt != nkt - 1:
                        continue
                    # evict accumulators fast so the PSUM banks free up,
                    # then normalize from SBUF
                    ssb = norm_pool.tile([HD, TC], f32, tag="ssb",
                                         name=f"ss{h}_{j}")
                    nc.vector.tensor_copy(ssb[:], ps_sum[h][:])
                    ysb = norm_pool.tile([HD, TC], f32, tag="ysb",
                                         name=f"ys{h}_{j}")
                    nc.vector.tensor_copy(ysb[:], ps_y[h][:])
                    rsum = norm_pool.tile([HD, TC], f32, tag="rsum",
                                          name=f"rs{h}_{j}")
                    nc.vector.reciprocal(rsum[:], ssb[:])
                    yt_t = yt_pool.tile([HD, TC], bf16, tag="yt",
                                        name=f"yt{h}_{j}")
                    nc.vector.tensor_mul(yt_t[:], ysb[:], rsum[:])
                    nc.gpsimd.dma_start(ag_in[j][HD * h:HD * (h + 1), :],
                                        yt_t[:])

        # ============ Phase C+D: AllGather chunk + output projection ============
        def phase_c(j):
            nc.gpsimd.collective_compute(
                "AllGather",
                mybir.AluOpType.bypass,
                replica_groups=[[0, 1, 2, 3], [4, 5, 6, 7]],
                ins=[ag_in[j].opt()],
                outs=[ag_out[j].opt()],
            )

        def phase_cd(j, wp, ytf_pool):
            ytf = []
            for ct in range(NCT):
                yf = ytf_pool.tile([128, TC], bf16, tag=f"ytf{ct}",
                                   name=f"ytf{ct}_{j}")
                nc.sync.dma_start(yf[:],
                                  ag_out[j][128 * ct:128 * (ct + 1), :])
                ytf.append(yf)
            for tt in range(NQ):
                ps = mm_psum.tile([128, TC], f32, tag="mm", name=f"o{j}_{tt}")
                for ct in range(NCT):
                    nc.tensor.matmul(
                        ps[:], ytf[ct][:, 128 * tt:128 * (tt + 1)], wp[ct][:],
                        start=(ct == 0), stop=(ct == NCT - 1))
                ot = o_sb_pool.tile([128, TC], f32, tag="ot",
                                    name=f"ot{j}_{tt}")
                nc.vector.tensor_copy(ot[:], ps[:])
                nc.gpsimd.dma_start(
                    out[TC * j + 128 * tt:TC * j + 128 * (tt + 1), :], ot[:])

        with tc.tile_pool(name="wq_pool", bufs=1) as wq_pool, \
             tc.tile_pool(name="xt_pool", bufs=2) as xt_pool, \
             tc.tile_pool(name="cs_pool", bufs=1) as cs_pool:
            cos_sb = cs_pool.tile([HD, T], f32)
            sin_sb = cs_pool.tile([HD, T], f32)
            nc.gpsimd.dma_start(cos_sb[:], cosT[:])
            nc.gpsimd.dma_start(sin_sb[:], sinT[:])
            wq = [wq_pool.tile([128, NR * HD], bf16, tag=f"wq{ct}",
                               name=f"wq{ct}")
                  for ct in range(NCT)]
            for ct in range(NCT):
                nc.scalar.dma_start(wq[ct][:],
                                    wqkvT[128 * ct:128 * (ct + 1), :])
            for j in range(NT):
                phase_a(j, wq, xt_pool, cos_sb, sin_sb)

        with tc.tile_pool(name="wp_pool", bufs=1) as wp_pool, \
             tc.tile_pool(name="ytf_pool", bufs=2) as ytf_pool:
            wp = [wp_pool.tile([128, TC], bf16, tag=f"wp{ct}", name=f"wp{ct}")
                  for ct in range(NCT)]
            for ct in range(NCT):
                nc.gpsimd.dma_start(wp[ct][:], wpT[128 * ct:128 * (ct + 1), :])
            phase_b(0)
            phase_c(0)
            phase_b(1)
            phase_c(1)
            phase_b(2)
            phase_c(2)
            phase_cd(0, wp, ytf_pool)
            phase_b(3)
            phase_c(3)
            phase_cd(1, wp, ytf_pool)
            phase_cd(2, wp, ytf_pool)
            phase_cd(3, wp, ytf_pool)


def _rope_tables():
    inv_freq = 1.0 / (ROPE_THETA ** (np.arange(0, HD, 2, dtype=np.float32) / HD))
    pos = np.arange(T, dtype=np.float32)
    freqs = pos[:, None] * inv_freq[None, :]
    emb = np.concatenate([freqs, freqs], axis=-1)          # [T, HD]
    cos = np.ascontiguousarray(np.cos(emb).astype(np.float32).T)   # [HD, T]
    sin = np.ascontiguousarray(np.sin(emb).astype(np.float32).T)
    sin[:64, :] *= -1.0                                    # sign for rotate_half
    return cos, sin


def _binmask():
    kk = np.arange(128)[:, None]
    qq = np.arange(TC)[None, :]
    blocks = [(kk <= qq - 128 * r).astype(np.float32) for r in range(G)]
    return np.ascontiguousarray(np.concatenate(blocks, axis=1))  # [128, 4*512]


def kernel(x, attention_mask, w_qkv, w_proj):
    from concourse.bass_utils import run_bass_kernel_spmd

    x = np.asarray(x, dtype=np.float32)
    attention_mask = np.asarray(attention_mask, dtype=np.float32)
    w_qkv = np.asarray(w_qkv, dtype=np.float32)
    w_proj = np.asarray(w_proj, dtype=np.float32)

    masked = not bool((attention_mask == 1.0).all())
    if masked:
        attention_mask = (attention_mask != 0.0).astype(np.float32)

    if masked not in _COMPILED:
        _COMPILED[masked] = _build(masked)
    nc = _COMPILED[masked]

    import ml_dtypes
    bf = ml_dtypes.bfloat16
    cos, sin = _rope_tables()
    bm = _binmask().astype(bf)
    wpT_full = np.ascontiguousarray(w_proj.T)              # [c, o]

    in_maps = []
    for c in range(N_CORES):
        b, g = divmod(c, G)
        xT_b = np.ascontiguousarray(x[b].T).astype(bf)
        rows_q = w_qkv[512 * g:512 * (g + 1)]
        rows_k = w_qkv[H * HD + HD * g:H * HD + HD * (g + 1)]
        rows_v = w_qkv[(H + KV) * HD + HD * g:(H + KV) * HD + HD * (g + 1)]
        wqkvT_g = np.ascontiguousarray(
            np.concatenate([rows_q, rows_k, rows_v], axis=0).T).astype(bf)
        m = {"xT": xT_b,
             "wqkvT": wqkvT_g,
             "wpT": np.ascontiguousarray(wpT_full[:, TC * g:TC * (g + 1)]),
             "cosT": cos, "sinT": sin, "binmask": bm}
        if masked:
            m["kmask"] = np.ascontiguousarray(
                attention_mask[b].reshape(NKT, 128).T)
        in_maps.append(m)

    trace = bool(globals().get("_TRACE", False))
    res = run_bass_kernel_spmd(nc, in_maps, core_ids=list(range(N_CORES)),
                               trace=trace)
    globals()["_LAST_RESULT"] = res

    y = np.empty((B, T, C), dtype=np.float32)
    for c in range(N_CORES):
        b, g = divmod(c, G)
        y[b, :, TC * g:TC * (g + 1)] = res.results[c]["out"]
    return y
